# revision 1
# baseline (speedup 1.0000x reference)
# HGNNP hypergraph convolution on 8 Trainium2 NeuronCores (Bass/Tile).
#
# Reference computation:
#   H      = relu(X @ W.T + b)                    [N, 128]
#   e_feat = segment_mean(H[v_idx], e_idx, E)     [E, 128]
#   out    = relu(segment_mean(e_feat[e_idx], v_idx, N))
#
# Strategy: DENSE block-matmul formulation (zero gather descriptors).
#   On this part, indexed-DMA (dma_gather / dma_scatter_add) is descriptor-
#   rate-bound at ~8 ns per 256B descriptor regardless of payload or memory
#   (measured), so any per-entry gather design costs >= 2*NNZ/8 * 8ns ~ 6.4 ms
#   per core.  Instead we materialize the incidence matrix A (0/1 counts) as
#   fp8 tiles on the host and stream it from HBM at bulk rate (~242 GB/s):
#     phase A:  esum_cm[c, e]  = sum_vb  H_blk[vb]^T     @ A1[vb, e-chunk]
#     phase B:  out_cm[c, v]   = sum_eb  efeat_blk[eb]^T @ A2[eb, v-chunk]
#   Both phases contract on the TensorEngine with the small dense operand
#   (H block / e_feat block, fp16) stationary and fp8 incidence chunks
#   streaming.  fp8 e4m3 holds small integer counts exactly, so the
#   segment sums are exact; per-core partial edge sums are AllReduced in
#   fp16 and scaled by 1/deg.  ~315 MB of A per phase per core at bulk HBM
#   rate ~= 1.3 ms/phase, matching the memory roofline for this regime.
import numpy as np

P = 128

N_VERTICES = 100000
N_EDGES = 25000
IN_CH = 256
OUT_CH = 128
N_CORES = 8

VPC = 12544              # vertices per core (98 blocks of 128)
VB = VPC // P            # 98
NV_PAD = N_CORES * VPC   # 100352
NE_PAD = 25088           # 196 blocks of 128
EB = NE_PAD // P         # 196

# phase A: 49 e-chunks of 512, in 7 passes x 7 chunks (7 PSUM banks)
ECW = 512
EPASS, ECHK = 7, 7       # 7*7*512 == 25088
# phase B: 25 v-chunks of 512 on 12800 padded rows, 5 passes x 5 chunks
VCW = 512
VPASS, VCHK = 5, 5       # 5*5*512 == 12800
VPC_B = VPASS * VCHK * VCW   # 12800
VBB = VPC_B // P         # 100 output blocks

_PROG_CACHE = {}
LAST_RESULTS = None      # BassKernelResults of the most recent run (for test.py)
LAST_IN_MAPS = None      # packed per-core inputs of the most recent kernel()


def build_program():
    import concourse.mybir as mybir
    import concourse.tile as tile
    from concourse import bacc

    dt = mybir.dt
    KC = IN_CH // P      # 2

    nc = bacc.Bacc("TRN2", target_bir_lowering=False, debug=False,
                   num_devices=N_CORES)

    # ---- I/O ----
    xt = nc.dram_tensor("xt", [IN_CH, VPC], dt.float16, kind="ExternalInput")
    wt = nc.dram_tensor("wt", [IN_CH, OUT_CH], dt.float16, kind="ExternalInput")
    bmat = nc.dram_tensor("bmat", [P, OUT_CH], dt.float32, kind="ExternalInput")
    ident = nc.dram_tensor("ident", [P, P], dt.float16, kind="ExternalInput")
    a1 = nc.dram_tensor("a1", [EPASS * VB * P, ECHK * ECW], dt.float8e4,
                        kind="ExternalInput")
    a2 = nc.dram_tensor("a2", [VPASS * EB * P, VCHK * VCW], dt.float8e4,
                        kind="ExternalInput")
    re_p = nc.dram_tensor("re", [P, EB], dt.float32, kind="ExternalInput")
    rv_p = nc.dram_tensor("rv", [P, VBB], dt.float32, kind="ExternalInput")
    out = nc.dram_tensor("out", [VPC_B, OUT_CH], dt.float32,
                         kind="ExternalOutput")

    # ---- internal DRAM ----
    # pass-major slabs so each pass's partial sums AllReduce independently,
    # overlapping the collective with later phase-A passes and phase B
    esum = nc.dram_tensor("esum", [EPASS, P, ECHK * ECW], dt.float16)
    esum_red = nc.dram_tensor("esum_red", [EPASS, P, ECHK * ECW], dt.float16,
                              addr_space="Shared")

    with tile.TileContext(nc) as tc:
        import contextlib
        with contextlib.ExitStack() as ctx:
            const = ctx.enter_context(tc.tile_pool(name="const", bufs=1))
            hpool = ctx.enter_context(tc.tile_pool(name="hpool", bufs=1))
            efpool = ctx.enter_context(tc.tile_pool(name="efpool", bufs=1))
            a1pool = ctx.enter_context(tc.tile_pool(name="a1pool", bufs=3))
            a2pool = ctx.enter_context(tc.tile_pool(name="a2pool", bufs=3))
            echpool = ctx.enter_context(tc.tile_pool(name="echpool", bufs=2))
            work = ctx.enter_context(tc.tile_pool(name="work", bufs=3))
            # 7 rotating PSUM slot names (one bank each); all stages share
            psA = ctx.enter_context(tc.tile_pool(name="psA", bufs=1,
                                                 space="PSUM"))

            # ---- constants ----
            xt_sb = const.tile([P, KC, VPC], dt.float16)
            for k in range(KC):
                nc.sync.dma_start(out=xt_sb[:, k, :], in_=xt[k * P:(k + 1) * P, :])
            wt_sb = const.tile([P, KC, OUT_CH], dt.float16)
            for k in range(KC):
                nc.sync.dma_start(out=wt_sb[:, k, :], in_=wt[k * P:(k + 1) * P, :])
            bb = const.tile([P, OUT_CH], dt.float32)
            nc.sync.dma_start(out=bb[:], in_=bmat[:, :])
            id_sb = const.tile([P, P], dt.float16)
            nc.sync.dma_start(out=id_sb[:], in_=ident[:, :])
            re_sb = const.tile([P, EB], dt.float32)
            nc.sync.dma_start(out=re_sb[:], in_=re_p[:, :])
            rv_sb = const.tile([P, VBB], dt.float32)
            nc.sync.dma_start(out=rv_sb[:], in_=rv_p[:, :])

            # ---- stage H: H = relu(X @ W.T + b), fp16 blocks in SBUF ----
            # h_sb[vr, vb, c] = H[vb*128+vr, c]
            h_sb = hpool.tile([P, VB, OUT_CH], dt.float16)
            for vb in range(VB):
                ps = psA.tile([P, OUT_CH], dt.float32, space="PSUM",
                              name=f"ps{vb % 4}")
                for k in range(KC):
                    nc.tensor.matmul(out=ps[:],
                                     lhsT=xt_sb[:, k, vb * P:(vb + 1) * P],
                                     rhs=wt_sb[:, k, :],
                                     start=(k == 0), stop=(k == KC - 1))
                tmp = work.tile([P, OUT_CH], dt.float32)
                nc.vector.tensor_add(out=tmp[:], in0=ps[:], in1=bb[:])
                nc.vector.tensor_scalar_max(out=h_sb[:, vb, :], in0=tmp[:],
                                            scalar1=0.0)

            # ---- phase A: esum_cm[c, e] = sum_vb H[vb]^T @ A1[vb, echunk] ----
            for sp in range(EPASS):
                pss = [psA.tile([P, ECW], dt.float32, space="PSUM",
                                name=f"ps{j}") for j in range(ECHK)]
                for vp in range(VB // 2):
                    a1t = a1pool.tile([P, 2, ECHK * ECW], dt.float8e4)
                    r0 = (sp * VB + 2 * vp) * P
                    nc.sync.dma_start(
                        out=a1t[:],
                        in_=a1[r0:r0 + 2 * P, :].rearrange("(b p) c -> p b c",
                                                           p=P))
                    for bi in range(2):
                        vb = 2 * vp + bi
                        for j in range(ECHK):
                            nc.tensor.matmul(out=pss[j][:],
                                             lhsT=h_sb[:, vb, :],
                                             rhs=a1t[:, bi,
                                                     j * ECW:(j + 1) * ECW],
                                             start=(vb == 0),
                                             stop=(vb == VB - 1))
                for j in range(ECHK):
                    es = work.tile([P, ECW], dt.float16)
                    nc.vector.tensor_copy(out=es[:], in_=pss[j][:])
                    nc.sync.dma_start(out=esum[sp, :, j * ECW:(j + 1) * ECW],
                                      in_=es[:])
                # per-pass AllReduce of this slab (overlaps later passes)
                nc.gpsimd.collective_compute(
                    "AllReduce", mybir.AluOpType.add,
                    replica_groups=[list(range(N_CORES))],
                    ins=[esum[sp].opt()], outs=[esum_red[sp].opt()])

            # ---- e_feat blocks: transpose each eb block, scale by 1/e_deg ----
            # ef_sb[er, eb, c] = esum_red[c, eb*128+er] * re[er, eb]
            ef_sb = efpool.tile([P, EB, OUT_CH], dt.float16)
            EBG = 14                         # eb blocks per bulk load
            for g in range(EB // EBG):
                ech = echpool.tile([P, EBG * P], dt.float16)
                sp_g = g * EBG // (ECHK * ECW // P)
                c0 = g * EBG * P - sp_g * ECHK * ECW
                nc.sync.dma_start(out=ech[:],
                                  in_=esum_red[sp_g, :, c0:c0 + EBG * P])
                for s in range(EBG):
                    eb = g * EBG + s
                    pst = psA.tile([P, P], dt.float16, space="PSUM",
                                   name=f"ps{s % 2}")
                    nc.tensor.transpose(pst[:], ech[:, s * P:(s + 1) * P],
                                        id_sb[:])
                    nc.vector.tensor_scalar_mul(out=ef_sb[:, eb, :], in0=pst[:],
                                                scalar1=re_sb[:, eb:eb + 1])

            # ---- phase B: out_cm[c, v] = sum_eb ef[eb]^T @ A2[eb, vchunk] ----
            for sp in range(VPASS):
                pss = [psA.tile([P, VCW], dt.float32, space="PSUM",
                                name=f"ps{j}") for j in range(VCHK)]
                for ep in range(EB // 2):
                    a2t = a2pool.tile([P, 2, VCHK * VCW], dt.float8e4)
                    r0 = (sp * EB + 2 * ep) * P
                    nc.sync.dma_start(
                        out=a2t[:],
                        in_=a2[r0:r0 + 2 * P, :].rearrange("(b p) c -> p b c",
                                                           p=P))
                    for bi in range(2):
                        eb = 2 * ep + bi
                        for j in range(VCHK):
                            nc.tensor.matmul(out=pss[j][:],
                                             lhsT=ef_sb[:, eb, :],
                                             rhs=a2t[:, bi,
                                                     j * VCW:(j + 1) * VCW],
                                             start=(eb == 0),
                                             stop=(eb == EB - 1))
                for j in range(VCHK):
                    cm = work.tile([P, VCW], dt.float16)
                    nc.vector.tensor_copy(out=cm[:], in_=pss[j][:])
                    for b in range(VCW // P):
                        vbb = (sp * VCHK + j) * (VCW // P) + b
                        pst = psA.tile([P, P], dt.float16, space="PSUM",
                                       name=f"ps{5 + b % 2}")
                        nc.tensor.transpose(pst[:], cm[:, b * P:(b + 1) * P],
                                            id_sb[:])
                        ot = work.tile([P, OUT_CH], dt.float32)
                        nc.vector.tensor_scalar(out=ot[:], in0=pst[:],
                                                scalar1=rv_sb[:, vbb:vbb + 1],
                                                scalar2=0.0,
                                                op0=mybir.AluOpType.mult,
                                                op1=mybir.AluOpType.max)
                        nc.sync.dma_start(out=out[vbb * P:(vbb + 1) * P, :],
                                          in_=ot[:])

    nc.compile()
    return nc


def pack_inputs(X, W, b, v_idx, e_idx):
    """Host-side preprocessing: build per-core fp8 incidence tiles in the
    pass/block-chunk layouts the device program streams, plus dense inputs."""
    import ml_dtypes
    f16, f32 = np.float16, np.float32
    f8 = ml_dtypes.float8_e4m3

    v = np.asarray(v_idx).astype(np.int64)
    e = np.asarray(e_idx).astype(np.int64)

    # fp8 byte LUT for small counts (0..15); counts beyond 15 are impossible
    # for random data but clip defensively (value error stays tiny/local).
    lut = np.arange(16, dtype=np.float32).astype(f8).view(np.uint8)

    # dense inputs
    xt_full = np.zeros((IN_CH, NV_PAD), f16)
    xt_full[:, :N_VERTICES] = np.asarray(X, f32).T.astype(f16)
    wt = np.ascontiguousarray(np.asarray(W, f32).T.astype(f16))
    bmat = np.tile(np.asarray(b, f32)[None, :], (P, 1))
    ident = np.eye(P, dtype=f16)

    # degree reciprocals
    edeg = np.bincount(e, minlength=NE_PAD).astype(f32)
    re = (1.0 / np.maximum(edeg, 1.0)).astype(f32)
    re_p = np.ascontiguousarray(re.reshape(EB, P).T)          # [er, eb]
    vdeg = np.bincount(v, minlength=N_CORES * VPC_B).astype(f32)
    rv = (1.0 / np.maximum(vdeg, 1.0)).astype(f32)

    core = v // VPC
    vl = v - core * VPC

    def counts_to_f8(cnt_u8, rows, cols):
        # fp8 e4m3 byte for 1.0 is 0x38; counts are overwhelmingly 0/1, so a
        # byte-multiply covers them and the rare multi-edges get LUT-fixed.
        out = cnt_u8 * np.uint8(0x38)
        fix = np.flatnonzero(cnt_u8 > 1)
        if fix.size:
            out[fix] = lut[np.minimum(cnt_u8[fix], 15)].copy()
        return out.view(f8).reshape(rows, cols)

    in_maps = []
    for c in range(N_CORES):
        m = core == c
        vc, ec = vl[m], e[m]

        # a1[(sp*VB+vb)*P + vr, ecp*ECW + el] = count(v==vb*P+vr,
        #                                             e==(sp*ECHK+ecp)*ECW+el)
        EC = ECHK * ECW
        row1 = (ec // EC * VB + vc // P) * P + vc % P
        cnt = np.zeros(EPASS * VB * P * EC, np.uint8)
        np.add.at(cnt, row1 * EC + ec % EC, 1)
        a1 = counts_to_f8(cnt, EPASS * VB * P, EC)
        del cnt

        # a2[(sp*EB+eb)*P + er, vcp*VCW + vl] = count(e==eb*P+er,
        #                                             v==(sp*VCHK+vcp)*VCW+vl)
        VC = VCHK * VCW
        row2 = (vc // VC * EB + ec // P) * P + ec % P
        cnt = np.zeros(VPASS * EB * P * VC, np.uint8)
        np.add.at(cnt, row2 * VC + vc % VC, 1)
        a2 = counts_to_f8(cnt, VPASS * EB * P, VC)
        del cnt

        rv_core = rv[c * VPC:(c + 1) * VPC]
        rv_pad = np.zeros(VPC_B, f32)
        rv_pad[:VPC] = rv_core
        in_maps.append({
            "xt": np.ascontiguousarray(xt_full[:, c * VPC:(c + 1) * VPC]),
            "wt": wt,
            "bmat": bmat,
            "ident": ident,
            "a1": a1,
            "a2": a2,
            "re": re_p,
            "rv": np.ascontiguousarray(rv_pad.reshape(VBB, P).T),
        })
    return in_maps


def run(in_maps, trace=False):
    global LAST_RESULTS
    from concourse.bass_utils import run_bass_kernel_spmd
    if "prog" not in _PROG_CACHE:
        _PROG_CACHE["prog"] = build_program()
    nc = _PROG_CACHE["prog"]
    res = run_bass_kernel_spmd(nc, in_maps, core_ids=list(range(N_CORES)),
                               trace=trace)
    LAST_RESULTS = res
    return res


def kernel(X, W, b, v_idx, e_idx, trace=False):
    global LAST_IN_MAPS
    in_maps = pack_inputs(X, W, b, v_idx, e_idx)
    LAST_IN_MAPS = in_maps
    res = run(in_maps, trace=trace)
    out = np.concatenate([res.results[c]["out"][:VPC] for c in range(N_CORES)],
                         axis=0)
    return np.ascontiguousarray(out[:N_VERTICES]).astype(np.float32)



# revision 3
# speedup vs baseline: 1.0648x; 1.0648x over previous
# HGNNP hypergraph convolution on 8 Trainium2 NeuronCores (Bass/Tile).
#
# Reference computation:
#   H      = relu(X @ W.T + b)                    [N, 128]
#   e_feat = segment_mean(H[v_idx], e_idx, E)     [E, 128]
#   out    = relu(segment_mean(e_feat[e_idx], v_idx, N))
#
# Strategy: DENSE block-matmul formulation (zero gather descriptors).
#   On this part, indexed-DMA (dma_gather / dma_scatter_add) is descriptor-
#   rate-bound at ~8 ns per 256B descriptor regardless of payload or memory
#   (measured), so any per-entry gather design costs >= 2*NNZ/8 * 8ns ~ 6.4 ms
#   per core.  Instead we materialize the incidence matrix A (0/1 counts) as
#   fp8 tiles on the host and stream it from HBM at bulk rate (~242 GB/s):
#     phase A:  esum_cm[c, e]  = sum_vb  H_blk[vb]^T     @ A1[vb, e-chunk]
#     phase B:  out_cm[c, v]   = sum_eb  efeat_blk[eb]^T @ A2[eb, v-chunk]
#   Both phases contract on the TensorEngine with the small dense operand
#   (H block / e_feat block, fp16) stationary and fp8 incidence chunks
#   streaming.  fp8 e4m3 holds small integer counts exactly, so the
#   segment sums are exact; per-core partial edge sums are AllReduced in
#   fp16 and scaled by 1/deg.  ~315 MB of A per phase per core at bulk HBM
#   rate ~= 1.3 ms/phase, matching the memory roofline for this regime.
import numpy as np

P = 128

N_VERTICES = 100000
N_EDGES = 25000
IN_CH = 256
OUT_CH = 128
N_CORES = 8

VPC = 12544              # vertices per core (98 blocks of 128)
VB = VPC // P            # 98
NV_PAD = N_CORES * VPC   # 100352
NE_PAD = 25088           # 196 blocks of 128
EB = NE_PAD // P         # 196

# phase A: 49 e-chunks of 512, in 7 passes x 7 chunks (7 PSUM banks)
ECW = 512
EPASS, ECHK = 7, 7       # 7*7*512 == 25088
# phase B: 25 v-chunks of 512 on 12800 padded rows, 5 passes x 5 chunks
VCW = 512
VPASS, VCHK = 5, 5       # 5*5*512 == 12800
VPC_B = VPASS * VCHK * VCW   # 12800
VBB = VPC_B // P         # 100 output blocks

_PROG_CACHE = {}
LAST_RESULTS = None      # BassKernelResults of the most recent run (for test.py)
LAST_IN_MAPS = None      # packed per-core inputs of the most recent kernel()


def build_program():
    import concourse.mybir as mybir
    import concourse.tile as tile
    from concourse import bacc

    dt = mybir.dt
    KC = IN_CH // P      # 2

    nc = bacc.Bacc("TRN2", target_bir_lowering=False, debug=False,
                   num_devices=N_CORES)

    # ---- I/O ----
    xt = nc.dram_tensor("xt", [IN_CH, VPC], dt.float16, kind="ExternalInput")
    wt = nc.dram_tensor("wt", [IN_CH, OUT_CH], dt.float16, kind="ExternalInput")
    bmat = nc.dram_tensor("bmat", [P, OUT_CH], dt.float32, kind="ExternalInput")
    ident = nc.dram_tensor("ident", [P, P], dt.float16, kind="ExternalInput")
    a1 = nc.dram_tensor("a1", [EPASS * VB * P, ECHK * ECW], dt.float8e4,
                        kind="ExternalInput")
    a2 = nc.dram_tensor("a2", [VPASS * EB * P, VCHK * VCW], dt.float8e4,
                        kind="ExternalInput")
    re_p = nc.dram_tensor("re", [P, EB], dt.float32, kind="ExternalInput")
    rv_p = nc.dram_tensor("rv", [P, VBB], dt.float32, kind="ExternalInput")
    out = nc.dram_tensor("out", [VPC_B, OUT_CH], dt.float32,
                         kind="ExternalOutput")

    # ---- internal DRAM ----
    # pass-major slabs so each pass's partial sums AllReduce independently,
    # overlapping the collective with later phase-A passes and phase B
    esum = nc.dram_tensor("esum", [EPASS, P, ECHK * ECW], dt.float16)
    esum_red = nc.dram_tensor("esum_red", [EPASS, P, ECHK * ECW], dt.float16,
                              addr_space="Shared")

    with tile.TileContext(nc) as tc:
        import contextlib
        with contextlib.ExitStack() as ctx:
            const = ctx.enter_context(tc.tile_pool(name="const", bufs=1))
            hpool = ctx.enter_context(tc.tile_pool(name="hpool", bufs=1))
            efpool = ctx.enter_context(tc.tile_pool(name="efpool", bufs=1))
            a1pool = ctx.enter_context(tc.tile_pool(name="a1pool", bufs=3))
            a2pool = ctx.enter_context(tc.tile_pool(name="a2pool", bufs=3))
            echpool = ctx.enter_context(tc.tile_pool(name="echpool", bufs=2))
            work = ctx.enter_context(tc.tile_pool(name="work", bufs=3))
            # 7 rotating PSUM slot names (one bank each); all stages share
            psA = ctx.enter_context(tc.tile_pool(name="psA", bufs=1,
                                                 space="PSUM"))

            # ---- constants ----
            xt_sb = const.tile([P, KC, VPC], dt.float16)
            for k in range(KC):
                nc.sync.dma_start(out=xt_sb[:, k, :], in_=xt[k * P:(k + 1) * P, :])
            wt_sb = const.tile([P, KC, OUT_CH], dt.float16)
            for k in range(KC):
                nc.sync.dma_start(out=wt_sb[:, k, :], in_=wt[k * P:(k + 1) * P, :])
            bb = const.tile([P, OUT_CH], dt.float32)
            nc.sync.dma_start(out=bb[:], in_=bmat[:, :])
            id_sb = const.tile([P, P], dt.float16)
            nc.sync.dma_start(out=id_sb[:], in_=ident[:, :])
            re_sb = const.tile([P, EB], dt.float32)
            nc.sync.dma_start(out=re_sb[:], in_=re_p[:, :])
            rv_sb = const.tile([P, VBB], dt.float32)
            nc.sync.dma_start(out=rv_sb[:], in_=rv_p[:, :])

            # ---- stage H: H = relu(X @ W.T + b), fp16 blocks in SBUF ----
            # h_sb[vr, vb, c] = H[vb*128+vr, c]
            h_sb = hpool.tile([P, VB, OUT_CH], dt.float16)
            for vb in range(VB):
                ps = psA.tile([P, OUT_CH], dt.float32, space="PSUM",
                              name=f"ps{vb % 4}")
                for k in range(KC):
                    nc.tensor.matmul(out=ps[:],
                                     lhsT=xt_sb[:, k, vb * P:(vb + 1) * P],
                                     rhs=wt_sb[:, k, :],
                                     start=(k == 0), stop=(k == KC - 1))
                tmp = work.tile([P, OUT_CH], dt.float32)
                nc.vector.tensor_add(out=tmp[:], in0=ps[:], in1=bb[:])
                nc.vector.tensor_scalar_max(out=h_sb[:, vb, :], in0=tmp[:],
                                            scalar1=0.0)

            # ---- phase A: esum_cm[c, e] = sum_vb H[vb]^T @ A1[vb, echunk] ----
            for sp in range(EPASS):
                pss = [psA.tile([P, ECW], dt.float32, space="PSUM",
                                name=f"ps{j}") for j in range(ECHK)]
                for vp in range(VB // 2):
                    a1t = a1pool.tile([P, 2, ECHK * ECW], dt.float8e4)
                    r0 = (sp * VB + 2 * vp) * P
                    nc.sync.dma_start(
                        out=a1t[:],
                        in_=a1[r0:r0 + 2 * P, :].rearrange("(b p) c -> p b c",
                                                           p=P))
                    for bi in range(2):
                        vb = 2 * vp + bi
                        for j in range(ECHK):
                            nc.tensor.matmul(out=pss[j][:],
                                             lhsT=h_sb[:, vb, :],
                                             rhs=a1t[:, bi,
                                                     j * ECW:(j + 1) * ECW],
                                             start=(vb == 0),
                                             stop=(vb == VB - 1))
                for j in range(ECHK):
                    es = work.tile([P, ECW], dt.float16)
                    nc.vector.tensor_copy(out=es[:], in_=pss[j][:])
                    nc.sync.dma_start(out=esum[sp, :, j * ECW:(j + 1) * ECW],
                                      in_=es[:])
                # per-pass AllReduce of this slab (overlaps later passes)
                nc.gpsimd.collective_compute(
                    "AllReduce", mybir.AluOpType.add,
                    replica_groups=[list(range(N_CORES))],
                    ins=[esum[sp].opt()], outs=[esum_red[sp].opt()])

            # ---- e_feat blocks: transpose each eb block, scale by 1/e_deg ----
            # ef_sb[er, eb, c] = esum_red[c, eb*128+er] * re[er, eb]
            ef_sb = efpool.tile([P, EB, OUT_CH], dt.float16)
            EBG = 14                         # eb blocks per bulk load
            for g in range(EB // EBG):
                ech = echpool.tile([P, EBG * P], dt.float16)
                sp_g = g * EBG // (ECHK * ECW // P)
                c0 = g * EBG * P - sp_g * ECHK * ECW
                nc.sync.dma_start(out=ech[:],
                                  in_=esum_red[sp_g, :, c0:c0 + EBG * P])
                for s in range(EBG):
                    eb = g * EBG + s
                    pst = psA.tile([P, P], dt.float16, space="PSUM",
                                   name=f"ps{s % 2}")
                    nc.tensor.transpose(pst[:], ech[:, s * P:(s + 1) * P],
                                        id_sb[:])
                    nc.vector.tensor_scalar_mul(out=ef_sb[:, eb, :], in0=pst[:],
                                                scalar1=re_sb[:, eb:eb + 1])

            # ---- phase B: out_cm[c, v] = sum_eb ef[eb]^T @ A2[eb, vchunk] ----
            for sp in range(VPASS):
                pss = [psA.tile([P, VCW], dt.float32, space="PSUM",
                                name=f"ps{j}") for j in range(VCHK)]
                for ep in range(EB // 2):
                    a2t = a2pool.tile([P, 2, VCHK * VCW], dt.float8e4)
                    r0 = (sp * EB + 2 * ep) * P
                    nc.sync.dma_start(
                        out=a2t[:],
                        in_=a2[r0:r0 + 2 * P, :].rearrange("(b p) c -> p b c",
                                                           p=P))
                    for bi in range(2):
                        eb = 2 * ep + bi
                        for j in range(VCHK):
                            nc.tensor.matmul(out=pss[j][:],
                                             lhsT=ef_sb[:, eb, :],
                                             rhs=a2t[:, bi,
                                                     j * VCW:(j + 1) * VCW],
                                             start=(eb == 0),
                                             stop=(eb == EB - 1))
                for j in range(VCHK):
                    cm = work.tile([P, VCW], dt.float16)
                    nc.vector.tensor_copy(out=cm[:], in_=pss[j][:])
                    for b in range(VCW // P):
                        vbb = (sp * VCHK + j) * (VCW // P) + b
                        pst = psA.tile([P, P], dt.float16, space="PSUM",
                                       name=f"ps{5 + b % 2}")
                        nc.tensor.transpose(pst[:], cm[:, b * P:(b + 1) * P],
                                            id_sb[:])
                        ot = work.tile([P, OUT_CH], dt.float32)
                        nc.vector.tensor_scalar(out=ot[:], in0=pst[:],
                                                scalar1=rv_sb[:, vbb:vbb + 1],
                                                scalar2=0.0,
                                                op0=mybir.AluOpType.mult,
                                                op1=mybir.AluOpType.max)
                        nc.sync.dma_start(out=out[vbb * P:(vbb + 1) * P, :],
                                          in_=ot[:])

    nc.compile()
    return nc


def pack_inputs(X, W, b, v_idx, e_idx):
    """Host-side preprocessing: build per-core fp8 incidence tiles in the
    pass/block-chunk layouts the device program streams, plus dense inputs."""
    import ml_dtypes
    f16, f32 = np.float16, np.float32
    f8 = ml_dtypes.float8_e4m3

    v = np.asarray(v_idx).astype(np.int64)
    e = np.asarray(e_idx).astype(np.int64)

    # fp8 byte LUT for small counts (0..15); counts beyond 15 are impossible
    # for random data but clip defensively (value error stays tiny/local).
    lut = np.arange(16, dtype=np.float32).astype(f8).view(np.uint8)

    # dense inputs
    xt_full = np.zeros((IN_CH, NV_PAD), f16)
    xt_full[:, :N_VERTICES] = np.asarray(X, f32).T.astype(f16)
    wt = np.ascontiguousarray(np.asarray(W, f32).T.astype(f16))
    bmat = np.tile(np.asarray(b, f32)[None, :], (P, 1))
    ident = np.eye(P, dtype=f16)

    # degree reciprocals
    edeg = np.bincount(e, minlength=NE_PAD).astype(f32)
    re = (1.0 / np.maximum(edeg, 1.0)).astype(f32)
    re_p = np.ascontiguousarray(re.reshape(EB, P).T)          # [er, eb]
    vdeg = np.bincount(v, minlength=N_CORES * VPC_B).astype(f32)
    rv = (1.0 / np.maximum(vdeg, 1.0)).astype(f32)

    core = v // VPC
    vl = v - core * VPC

    def idx_to_f8(idx, rows, cols):
        # Build the fp8 0/1-count matrix directly: scatter the fp8 byte for
        # 1.0 (0x38), then LUT-fix the rare duplicate (v,e) pairs found via
        # np.unique.  Avoids np.add.at over the 300MB dense array.
        out = np.zeros(rows * cols, np.uint8)
        out[idx] = np.uint8(0x38)
        u, c = np.unique(idx, return_counts=True)
        dup = c > 1
        if dup.any():
            out[u[dup]] = lut[np.minimum(c[dup], 15)]
        return out.view(f8).reshape(rows, cols)

    in_maps = []
    for c in range(N_CORES):
        m = core == c
        vc, ec = vl[m], e[m]

        # a1[(sp*VB+vb)*P + vr, ecp*ECW + el] = count(v==vb*P+vr,
        #                                             e==(sp*ECHK+ecp)*ECW+el)
        EC = ECHK * ECW
        row1 = (ec // EC * VB + vc // P) * P + vc % P
        a1 = idx_to_f8(row1 * EC + ec % EC, EPASS * VB * P, EC)

        # a2[(sp*EB+eb)*P + er, vcp*VCW + vl] = count(e==eb*P+er,
        #                                             v==(sp*VCHK+vcp)*VCW+vl)
        VC = VCHK * VCW
        row2 = (vc // VC * EB + ec // P) * P + ec % P
        a2 = idx_to_f8(row2 * VC + vc % VC, VPASS * EB * P, VC)

        rv_core = rv[c * VPC:(c + 1) * VPC]
        rv_pad = np.zeros(VPC_B, f32)
        rv_pad[:VPC] = rv_core
        in_maps.append({
            "xt": np.ascontiguousarray(xt_full[:, c * VPC:(c + 1) * VPC]),
            "wt": wt,
            "bmat": bmat,
            "ident": ident,
            "a1": a1,
            "a2": a2,
            "re": re_p,
            "rv": np.ascontiguousarray(rv_pad.reshape(VBB, P).T),
        })
    return in_maps


def run(in_maps, trace=False):
    global LAST_RESULTS
    from concourse.bass_utils import run_bass_kernel_spmd
    if "prog" not in _PROG_CACHE:
        _PROG_CACHE["prog"] = build_program()
    nc = _PROG_CACHE["prog"]
    res = run_bass_kernel_spmd(nc, in_maps, core_ids=list(range(N_CORES)),
                               trace=trace)
    LAST_RESULTS = res
    return res


def kernel(X, W, b, v_idx, e_idx, trace=False):
    global LAST_IN_MAPS
    in_maps = pack_inputs(X, W, b, v_idx, e_idx)
    LAST_IN_MAPS = in_maps
    res = run(in_maps, trace=trace)
    out = np.concatenate([res.results[c]["out"][:VPC] for c in range(N_CORES)],
                         axis=0)
    return np.ascontiguousarray(out[:N_VERTICES]).astype(np.float32)



# revision 4
# speedup vs baseline: 1.8320x; 1.7205x over previous
# HGNNP hypergraph conv, v2: edge-sharded phase A, zero reduce collectives.
#
#   H      = relu(X @ W.T + b)                    [N, 128]
#   e_feat = segment_mean(H[v_idx], e_idx, E)     [E, 128]
#   out    = relu(segment_mean(e_feat[e_idx], v_idx, N))
#
# vs baseline (vertex-sharded phase A + 7x AllReduce of 6.4MB esum slabs):
#   - X is replicated; each core computes all H blocks on the fly (84us of
#     TensorE) and contracts them against the incidence columns of its OWN
#     edge slice (E/8 = 3125 edges, padded 3200).  Phase A emits a complete
#     0.8MB e_feat slice per core -> ONE AllGather (6.55MB out) replaces the
#     7 AllReduces.  Phase B (vertex-sharded) unchanged in structure.
#   - a1/a2/xt are host-pre-swizzled so every bulk DMA is 128 contiguous
#     per-partition descriptors (no (b p) c rearrange descriptor storms).
#   - e_feat slice is written [er, eb_local*128+c] so the AllGather output
#     loads into lhsT layout with 8 big descriptors per partition.
import numpy as np

P = 128

N_VERTICES = 100000
N_EDGES = 25000
IN_CH = 256
OUT_CH = 128
N_CORES = 8

NV_PAD = 100352          # 784 vertex blocks of 128
VB_ALL = NV_PAD // P     # 784
VGRP = VB_ALL // 4       # 196 groups of 4 blocks (a1/xt DMA granularity)

ES_REAL = N_EDGES // N_CORES   # 3125 edges per core slice
ES = 3200                      # padded slice width (25 blocks)
EBS = ES // P                  # 25 local e_feat blocks
EB_ALL = N_CORES * EBS         # 200 global blocks
# phase A psum chunks over the 3200 slice cols: 6x512 + 1x128 (banks 0-6)
A_CHUNKS = [(k * 512, 512) for k in range(6)] + [(3072, 128)]

VPC = 12544              # vertices per core (real rows used)
VPC_B = 12800            # padded to 100 output blocks
VBB = VPC_B // P         # 100
VPASS, VCHK, VCW = 5, 5, 512   # 5 passes x 5 chunks x 512 = 12800
EGRP = EB_ALL // 4       # 50 groups of 4 e-blocks per pass (a2 granularity)

_PROG_CACHE = {}
LAST_RESULTS = None
LAST_IN_MAPS = None


def build_program():
    import os
    import concourse.mybir as mybir
    import concourse.tile as tile
    from concourse import bacc

    dt = mybir.dt
    KC = IN_CH // P      # 2
    # timing-decomposition knobs (default 1 = production): repeat a phase
    # twice and the steady-state delta vs rep=1 is that phase's device time
    # (the dispatch floor cancels).
    rep_a = int(os.environ.get("V2_REP_A", "1"))
    rep_b = int(os.environ.get("V2_REP_B", "1"))

    nc = bacc.Bacc("TRN2", target_bir_lowering=False, debug=False,
                   num_devices=N_CORES)

    # ---- I/O ----
    # xt[p, g*1024 + k*512 + j] = X[g*512+j, k*128+p]  (replicated)
    xt = nc.dram_tensor("xt", [P, VGRP * KC * 512], dt.float16,
                        kind="ExternalInput")
    wt = nc.dram_tensor("wt", [IN_CH, OUT_CH], dt.float16,
                        kind="ExternalInput")
    bmat = nc.dram_tensor("bmat", [P, OUT_CH], dt.float32,
                          kind="ExternalInput")
    ident = nc.dram_tensor("ident", [P, P], dt.float16, kind="ExternalInput")
    # a1[p, ((g*4)+b)*3200 + col] = count(v == (4g+b)*128+p, e == slice col)
    a1 = nc.dram_tensor("a1", [P, VGRP * 4 * ES], dt.float8e4,
                        kind="ExternalInput")
    # a2[er, (((sp*50+ep)*4+b)*2560 + vcol] = count(e -> block 4ep+b row er,
    #                                               v == sp*2560+vcol local)
    a2 = nc.dram_tensor("a2", [P, VPASS * EGRP * 4 * (VCHK * VCW)],
                        dt.float8e4, kind="ExternalInput")
    re_p = nc.dram_tensor("re", [P, EBS], dt.float32, kind="ExternalInput")
    rv_p = nc.dram_tensor("rv", [P, VBB], dt.float32, kind="ExternalInput")
    out = nc.dram_tensor("out", [VPC_B, OUT_CH], dt.float32,
                         kind="ExternalOutput")

    # ---- internal DRAM ----
    esl = nc.dram_tensor("esl", [P, ES], dt.float16)           # local slice
    efg = nc.dram_tensor("efg", [N_CORES * P, ES], dt.float16,
                         addr_space="Shared")                  # gathered
    esl2 = (nc.dram_tensor("esl2", [P, ES], dt.float16)
            if rep_a > 1 else None)
    out2 = (nc.dram_tensor("out2", [VPC_B, OUT_CH], dt.float32)
            if rep_b > 1 else None)

    with tile.TileContext(nc) as tc:
        import contextlib
        with contextlib.ExitStack() as ctx:
            const = ctx.enter_context(tc.tile_pool(name="const", bufs=1))
            xpool = ctx.enter_context(tc.tile_pool(name="xpool", bufs=3))
            hpool = ctx.enter_context(tc.tile_pool(name="hpool", bufs=4))
            efpool = ctx.enter_context(tc.tile_pool(name="efpool", bufs=1))
            a1pool = ctx.enter_context(tc.tile_pool(name="a1pool", bufs=3))
            a2pool = ctx.enter_context(tc.tile_pool(name="a2pool", bufs=3))
            work = ctx.enter_context(tc.tile_pool(name="work", bufs=3))
            psA = ctx.enter_context(tc.tile_pool(name="psA", bufs=1,
                                                 space="PSUM"))

            # ---- constants ----
            wt_sb = const.tile([P, KC, OUT_CH], dt.float16)
            for k in range(KC):
                nc.sync.dma_start(out=wt_sb[:, k, :],
                                  in_=wt[k * P:(k + 1) * P, :])
            bb = const.tile([P, OUT_CH], dt.float32)
            nc.sync.dma_start(out=bb[:], in_=bmat[:, :])
            id_sb = const.tile([P, P], dt.float16)
            nc.sync.dma_start(out=id_sb[:], in_=ident[:, :])
            re_sb = const.tile([P, EBS], dt.float32)
            nc.sync.dma_start(out=re_sb[:], in_=re_p[:, :])
            rv_sb = const.tile([P, VBB], dt.float32)
            nc.sync.dma_start(out=rv_sb[:], in_=rv_p[:, :])

            # ---- phase A: esum[c, col] = sum_v H[v, c] * A1[v, col] ----
            # H blocks computed on the fly (ps7), one-block software pipeline
            # so phase-A matmuls for vb overlap the DVE relu of vb+1.
            def emit_phase_a(dst):
                pss = [psA.tile([P, w], dt.float32, space="PSUM",
                                name=f"ps{j}")
                       for j, (c0, w) in enumerate(A_CHUNKS)]
                pending = None   # (ht, a1 tile, block-in-group, vb)
                for g in range(VGRP):
                    xt_t = xpool.tile([P, KC * 512], dt.float16)
                    nc.sync.dma_start(out=xt_t[:],
                                      in_=xt[:, g * (KC * 512):
                                             (g + 1) * (KC * 512)])
                    a1t = a1pool.tile([P, 4 * ES], dt.float8e4)
                    nc.sync.dma_start(out=a1t[:],
                                      in_=a1[:, g * 4 * ES:(g + 1) * 4 * ES])
                    for i in range(4):
                        vb = 4 * g + i
                        hps = psA.tile([P, OUT_CH], dt.float32, space="PSUM",
                                       name="ps7")
                        for k in range(KC):
                            nc.tensor.matmul(
                                out=hps[:],
                                lhsT=xt_t[:, k * 512 + i * P:
                                          k * 512 + (i + 1) * P],
                                rhs=wt_sb[:, k, :],
                                start=(k == 0), stop=(k == KC - 1))
                        tmp = work.tile([P, OUT_CH], dt.float32)
                        nc.vector.tensor_add(out=tmp[:], in0=hps[:],
                                             in1=bb[:])
                        ht = hpool.tile([P, OUT_CH], dt.float16)
                        nc.vector.tensor_scalar_max(out=ht[:], in0=tmp[:],
                                                    scalar1=0.0)
                        if pending is not None:
                            pht, pa1t, pi, pvb = pending
                            for j, (c0, w) in enumerate(A_CHUNKS):
                                nc.tensor.matmul(
                                    out=pss[j][:],
                                    lhsT=pht[:],
                                    rhs=pa1t[:, pi * ES + c0:
                                             pi * ES + c0 + w],
                                    start=(pvb == 0),
                                    stop=(pvb == VB_ALL - 1))
                        pending = (ht, a1t, i, vb)
                pht, pa1t, pi, pvb = pending
                for j, (c0, w) in enumerate(A_CHUNKS):
                    nc.tensor.matmul(out=pss[j][:], lhsT=pht[:],
                                     rhs=pa1t[:, pi * ES + c0:
                                              pi * ES + c0 + w],
                                     start=(pvb == 0),
                                     stop=(pvb == VB_ALL - 1))

                # e_feat slice: [c, col] -> [er, ebl, c], scale 1/e_deg
                es_sb = efpool.tile([P, ES], dt.float16)
                for j, (c0, w) in enumerate(A_CHUNKS):
                    nc.vector.tensor_copy(out=es_sb[:, c0:c0 + w],
                                          in_=pss[j][:])
                ef_t = efpool.tile([P, EBS, OUT_CH], dt.float16)
                for ebl in range(EBS):
                    pst = psA.tile([P, P], dt.float16, space="PSUM",
                                   name=f"ps{ebl % 2}")
                    nc.tensor.transpose(pst[:],
                                        es_sb[:, ebl * P:(ebl + 1) * P],
                                        id_sb[:])
                    nc.vector.tensor_scalar_mul(out=ef_t[:, ebl, :],
                                                in0=pst[:],
                                                scalar1=re_sb[:, ebl:ebl + 1])
                nc.sync.dma_start(out=dst[:, :], in_=ef_t[:])

            for r in range(rep_a):
                emit_phase_a(esl if r == rep_a - 1 else esl2)

            # ---- AllGather the e_feat slices (0.8MB in, 6.55MB out) ----
            nc.gpsimd.collective_compute(
                "AllGather", mybir.AluOpType.bypass,
                replica_groups=[list(range(N_CORES))],
                ins=[esl[:, :].opt()], outs=[efg[:, :].opt()])

            # ef_sb[er, ct, ebl*128+c] = e_feat block (ct*25+ebl) as [er, c]
            ef_sb = efpool.tile([P, N_CORES, ES], dt.float16)
            nc.sync.dma_start(
                out=ef_sb[:],
                in_=efg.rearrange("(g p) c -> p g c", p=P))

            # ---- phase B: out_cm[c, vcol] = sum_r ef[r, c] * A2[r, vcol] ----
            def emit_phase_b(dst):
                for sp in range(VPASS):
                    psb = [psA.tile([P, VCW], dt.float32, space="PSUM",
                                    name=f"ps{j}") for j in range(VCHK)]
                    for ep in range(EGRP):
                        a2t = a2pool.tile([P, 4 * VCHK * VCW], dt.float8e4)
                        c0 = (sp * EGRP + ep) * 4 * VCHK * VCW
                        nc.sync.dma_start(out=a2t[:],
                                          in_=a2[:, c0:c0 + 4 * VCHK * VCW])
                        for bi in range(4):
                            rb = 4 * ep + bi
                            ct, ebl = rb // EBS, rb % EBS
                            for j in range(VCHK):
                                nc.tensor.matmul(
                                    out=psb[j][:],
                                    lhsT=ef_sb[:, ct, ebl * P:(ebl + 1) * P],
                                    rhs=a2t[:, (bi * VCHK + j) * VCW:
                                            (bi * VCHK + j + 1) * VCW],
                                    start=(rb == 0), stop=(rb == EB_ALL - 1))
                    for j in range(VCHK):
                        cm = work.tile([P, VCW], dt.float16)
                        nc.vector.tensor_copy(out=cm[:], in_=psb[j][:])
                        for bobj in range(VCW // P):
                            vbb = (sp * (VCHK * VCW // P)
                                   + j * (VCW // P) + bobj)
                            pst = psA.tile([P, P], dt.float16, space="PSUM",
                                           name=f"ps{5 + bobj % 2}")
                            nc.tensor.transpose(
                                pst[:], cm[:, bobj * P:(bobj + 1) * P],
                                id_sb[:])
                            ot = work.tile([P, OUT_CH], dt.float32)
                            nc.vector.tensor_scalar(
                                out=ot[:], in0=pst[:],
                                scalar1=rv_sb[:, vbb:vbb + 1],
                                scalar2=0.0,
                                op0=mybir.AluOpType.mult,
                                op1=mybir.AluOpType.max)
                            nc.sync.dma_start(
                                out=dst[vbb * P:(vbb + 1) * P, :],
                                in_=ot[:])

            for r in range(rep_b):
                emit_phase_b(out if r == rep_b - 1 else out2)

    nc.compile()
    return nc


def pack_inputs(X, W, b, v_idx, e_idx):
    import ml_dtypes
    f16, f32 = np.float16, np.float32
    f8 = ml_dtypes.float8_e4m3

    v = np.asarray(v_idx).astype(np.int64)
    e = np.asarray(e_idx).astype(np.int64)

    lut = np.arange(16, dtype=np.float32).astype(f8).view(np.uint8)

    def idx_to_f8(idx, rows, cols):
        out = np.zeros(rows * cols, np.uint8)
        out[idx] = np.uint8(0x38)        # fp8 e4m3 byte for 1.0
        u, c = np.unique(idx, return_counts=True)
        dup = c > 1
        if dup.any():
            out[u[dup]] = lut[np.minimum(c[dup], 15)]
        return out.view(f8).reshape(rows, cols)

    # xt[p, g, k, j] = X[g*512+j, k*128+p], replicated
    Xp = np.zeros((NV_PAD, IN_CH), f16)
    Xp[:N_VERTICES] = np.asarray(X, f32).astype(f16)
    xt = np.ascontiguousarray(
        Xp.reshape(VGRP, 512, 2, P).transpose(3, 0, 2, 1)
    ).reshape(P, VGRP * 2 * 512)
    wt = np.ascontiguousarray(np.asarray(W, f32).T.astype(f16))
    bmat = np.tile(np.asarray(b, f32)[None, :], (P, 1))
    identm = np.eye(P, dtype=f16)

    edeg = np.bincount(e, minlength=N_EDGES).astype(f32)
    er_all = 1.0 / np.maximum(edeg, 1.0)
    vdeg = np.bincount(v, minlength=N_CORES * VPC).astype(f32)
    rv_all = 1.0 / np.maximum(vdeg, 1.0)

    ecore = e // ES_REAL
    vcore = v // VPC

    in_maps = []
    for c in range(N_CORES):
        # a1: this core's edge slice, all vertices
        m = ecore == c
        vv, col = v[m], e[m] - c * ES_REAL
        p = vv % P
        vb = vv // P
        idx = ((p * VGRP + vb // 4) * 4 + vb % 4) * ES + col
        a1 = idx_to_f8(idx, P, VGRP * 4 * ES)

        # a2: this core's vertex slice, all edges (gathered-row indexing)
        m = vcore == c
        vl, ee = v[m] - c * VPC, e[m]
        sp, vcol = vl // (VCHK * VCW), vl % (VCHK * VCW)
        ecol = ee % ES_REAL
        bg = (ee // ES_REAL) * EBS + ecol // P
        er = ecol % P
        idx = (((er * VPASS + sp) * EGRP + bg // 4) * 4 + bg % 4) \
            * (VCHK * VCW) + vcol
        a2 = idx_to_f8(idx, P, VPASS * EGRP * 4 * VCHK * VCW)

        # re[er, ebl] for slice cols, pad cols -> 1.0
        re_slice = np.ones(ES, f32)
        re_slice[:ES_REAL] = er_all[c * ES_REAL:(c + 1) * ES_REAL]
        re_pm = np.ascontiguousarray(re_slice.reshape(EBS, P).T)

        rv_pad = np.zeros(VPC_B, f32)
        rv_pad[:VPC] = rv_all[c * VPC:(c + 1) * VPC]
        rv_pm = np.ascontiguousarray(rv_pad.reshape(VBB, P).T)

        in_maps.append({
            "xt": xt,
            "wt": wt,
            "bmat": bmat,
            "ident": identm,
            "a1": a1,
            "a2": a2,
            "re": re_pm,
            "rv": rv_pm,
        })
    return in_maps


def run(in_maps, trace=False):
    global LAST_RESULTS
    from concourse.bass_utils import run_bass_kernel_spmd
    if "prog" not in _PROG_CACHE:
        _PROG_CACHE["prog"] = build_program()
    nc = _PROG_CACHE["prog"]
    res = run_bass_kernel_spmd(nc, in_maps, core_ids=list(range(N_CORES)),
                               trace=trace)
    LAST_RESULTS = res
    return res


def kernel(X, W, b, v_idx, e_idx, trace=False):
    global LAST_IN_MAPS
    in_maps = pack_inputs(X, W, b, v_idx, e_idx)
    LAST_IN_MAPS = in_maps
    res = run(in_maps, trace=trace)
    out = np.concatenate([res.results[c]["out"][:VPC]
                          for c in range(N_CORES)], axis=0)
    return np.ascontiguousarray(out[:N_VERTICES]).astype(np.float32)
